# revision 3
# baseline (speedup 1.0000x reference)
"""Trainium2 Bass kernel for a GPT-2-style transformer block.

B=1, T=4096, C=768, H=12 heads (hd=64), causal attention, exact GELU MLP.

Distribution over 8 NeuronCores (single shared SPMD program):
  - LN1+QKV: each core computes K^T/V for its contiguous 512-token slab,
    AllGathers K/V (bf16) so every core sees the full sequence.
  - Queries: mod-8 interleaved sharding (core c owns tokens t with t%8==c).
    This makes the causal-attention instruction structure identical on all
    cores (required: one program, per-core behavior only via input data).
    Per-core masks for the diagonal band are fed as inputs.
  - proj/LN2/MLP/residual: row-parallel on each core's own query rows.
  - Host: shards/transposes/casts inputs, reassembles the output.

All activations live in transposed [feature, token] layout on-chip; LN
statistics use ones-matmul partition reductions; softmax row-sums use
col-tiled ones-matmuls; matmul inputs are bf16 with fp32 accumulation and
an fp32 residual stream.
"""

import numpy as np
import ml_dtypes

import concourse.bacc as bacc
import concourse.mybir as mybir
import concourse.tile as tile
from concourse.bass_utils import run_bass_kernel_spmd

BF16 = ml_dtypes.bfloat16

# problem shape (hardcoded per harness contract)
T = 4096
C = 768
H = 12
HD = 64
EPS = 1e-5
NC = 8          # cores
R = 512         # tokens per core
P = 128
CT = C // P     # 6 feature tiles
QT = R // P     # 4 query tiles per core
KT = T // P     # 32 key tiles
PAIRS = H // 2  # 6 head pairs
HT = (4 * C) // P  # 24 hidden tiles
KV_K = C * R    # bf16 elems of K^T slab
KV_N = KV_K + R * C  # + V slab

_CACHE = {}


def _ln_transposed(nc, tc, pool, pspool, xT, out_bf, ones_sb, w_col, b_col, apply_wb):
    """LayerNorm over the feature axis for [C, R]-transposed activations.

    xT: f32 sbuf tile [P, CT, R]; out_bf: bf16 sbuf tile [P, CT, R].
    Stats via ones-matmul partition reduction (all-partition-broadcast
    results), rstd = exp(-0.5*ln(var+eps)).
    """
    f32 = mybir.dt.float32
    bf16 = mybir.dt.bfloat16
    xb = pool.tile([P, CT, R], bf16, tag="ln_xb")
    sq = pool.tile([P, CT, R], bf16, tag="ln_sq")
    nc.vector.tensor_copy(xb[:], xT[:])
    nc.vector.tensor_mul(sq[:], xb[:], xb[:])
    ps_sum = pspool.tile([P, R], f32, tag="ln_psum")
    ps_sq = pspool.tile([P, R], f32, tag="ln_pssq")
    for k in range(CT):
        nc.tensor.matmul(ps_sum[:], ones_sb[:], xb[:, k, :], start=(k == 0), stop=(k == CT - 1))
    for k in range(CT):
        nc.tensor.matmul(ps_sq[:], ones_sb[:], sq[:, k, :], start=(k == 0), stop=(k == CT - 1))
    nmean = pool.tile([P, R], f32, tag="ln_nmean")
    m2 = pool.tile([P, R], f32, tag="ln_m2")
    nc.vector.tensor_scalar_mul(nmean[:], ps_sum[:], -1.0 / C)
    nc.vector.tensor_scalar_mul(m2[:], ps_sq[:], 1.0 / C)
    var = pool.tile([P, R], f32, tag="ln_var")
    nc.vector.tensor_mul(var[:], nmean[:], nmean[:])          # mean^2
    # var = (E[x^2] + eps) - mean^2
    nc.vector.scalar_tensor_tensor(
        var[:], m2[:], EPS, var[:], mybir.AluOpType.add, mybir.AluOpType.subtract
    )
    rstd = pool.tile([P, R], f32, tag="ln_rstd")
    nc.scalar.activation(var[:], var[:], mybir.ActivationFunctionType.Ln)
    nc.scalar.activation(rstd[:], var[:], mybir.ActivationFunctionType.Exp, scale=-0.5)
    nmr = pool.tile([P, R], f32, tag="ln_nmr")
    nc.vector.tensor_mul(nmr[:], nmean[:], rstd[:])           # -mu*rstd
    tmp = pool.tile([P, R], f32, tag="ln_tmp")
    for k in range(CT):
        nc.vector.tensor_mul(tmp[:], xT[:, k, :], rstd[:])
        if apply_wb:
            nc.vector.tensor_add(tmp[:], tmp[:], nmr[:])
            nc.vector.tensor_scalar(
                out_bf[:, k, :], tmp[:], w_col[:, k : k + 1], b_col[:, k : k + 1],
                mybir.AluOpType.mult, mybir.AluOpType.add,
            )
        else:
            nc.vector.tensor_add(out_bf[:, k, :], tmp[:], nmr[:])


def _build(apply_ln1, apply_ln2, apply_bv):
    key = (apply_ln1, apply_ln2, apply_bv)
    if key in _CACHE:
        return _CACHE[key]

    f32 = mybir.dt.float32
    bf16 = mybir.dt.bfloat16
    AF = mybir.ActivationFunctionType

    nc = bacc.Bacc("TRN2", target_bir_lowering=False, debug=False, num_devices=NC)

    xTs_d = nc.declare_dram_parameter("xTs", [C, R], f32, isOutput=False)
    xTq_d = nc.declare_dram_parameter("xTq", [C, R], f32, isOutput=False)
    masks_d = nc.declare_dram_parameter("masks", [P, 8, P], bf16, isOutput=False)
    ones_d = nc.declare_dram_parameter("ones", [P, P], bf16, isOutput=False)
    wattn_d = nc.declare_dram_parameter("wattn", [C, 3 * C], bf16, isOutput=False)
    wproj_d = nc.declare_dram_parameter("wproj", [C, C], bf16, isOutput=False)
    wfc_d = nc.declare_dram_parameter("wfc", [C, 4 * C], bf16, isOutput=False)
    wfc2_d = nc.declare_dram_parameter("wfc2", [4 * C, C], bf16, isOutput=False)
    bqk_d = nc.declare_dram_parameter("bqk", [P, 2 * CT], f32, isOutput=False)
    bproj_d = nc.declare_dram_parameter("bproj", [P, CT], f32, isOutput=False)
    bfc_d = nc.declare_dram_parameter("bfc", [P, HT], f32, isOutput=False)
    bfc2_d = nc.declare_dram_parameter("bfc2", [P, CT], f32, isOutput=False)
    if apply_bv:
        bv_d = nc.declare_dram_parameter("bv", [P, C], f32, isOutput=False)
    if apply_ln1:
        ln1w_d = nc.declare_dram_parameter("ln1w", [P, CT], f32, isOutput=False)
        ln1b_d = nc.declare_dram_parameter("ln1b", [P, CT], f32, isOutput=False)
    if apply_ln2:
        ln2w_d = nc.declare_dram_parameter("ln2w", [P, CT], f32, isOutput=False)
        ln2b_d = nc.declare_dram_parameter("ln2b", [P, CT], f32, isOutput=False)
    outT_d = nc.declare_dram_parameter("outT", [C, R], f32, isOutput=True)

    with tile.TileContext(nc) as tc:
        with (
            tc.tile_pool(name="const", bufs=1) as const,
            tc.tile_pool(name="dram", bufs=1, space="DRAM") as dram,
            tc.tile_pool(name="mid", bufs=1) as mid,
        ):
            ones_sb = const.tile([P, P], bf16)
            nc.sync.dma_start(ones_sb[:], ones_d[:])
            masks_sb = const.tile([P, 8, P], bf16)
            nc.sync.dma_start(masks_sb[:], masks_d[:])
            wproj_sb = const.tile([P, CT, C], bf16)
            nc.sync.dma_start(wproj_sb[:], wproj_d.rearrange("(o p) f -> p o f", p=P))
            bqk_sb = const.tile([P, 2 * CT], f32)
            nc.sync.dma_start(bqk_sb[:], bqk_d[:])
            bproj_sb = const.tile([P, CT], f32)
            nc.sync.dma_start(bproj_sb[:], bproj_d[:])
            bfc_sb = const.tile([P, HT], f32)
            nc.sync.dma_start(bfc_sb[:], bfc_d[:])
            bfc2_sb = const.tile([P, CT], f32)
            nc.sync.dma_start(bfc2_sb[:], bfc2_d[:])
            if apply_bv:
                bv_sb = const.tile([P, C], f32)
                nc.sync.dma_start(bv_sb[:], bv_d[:])
            ln1w_sb = ln1b_sb = ln2w_sb = ln2b_sb = None
            if apply_ln1:
                ln1w_sb = const.tile([P, CT], f32)
                ln1b_sb = const.tile([P, CT], f32)
                nc.sync.dma_start(ln1w_sb[:], ln1w_d[:])
                nc.sync.dma_start(ln1b_sb[:], ln1b_d[:])
            if apply_ln2:
                ln2w_sb = const.tile([P, CT], f32)
                ln2b_sb = const.tile([P, CT], f32)
                nc.sync.dma_start(ln2w_sb[:], ln2w_d[:])
                nc.sync.dma_start(ln2b_sb[:], ln2b_d[:])
            xTq_sb = const.tile([P, CT, R], f32)
            nc.sync.dma_start(xTq_sb[:], xTq_d.rearrange("(o p) t -> p o t", p=P))

            # mid-lifetime tiles
            q_sb = mid.tile([P, CT, R], bf16)      # Q^T for own queries
            ynorm_sb = mid.tile([P, CT, R], bf16)  # normalized attn out (y^T)
            z_sb = mid.tile([P, CT, R], f32)       # residual stream x+attn (z^T)
            xln2_sb = mid.tile([P, CT, R], bf16)
            wfc_sb = mid.tile([P, CT, 4 * C], bf16)

            kv_local = dram.tile([1, KV_N], bf16)
            kv_all = dram.tile([NC, KV_N], bf16)
            kvl_k = kv_local[0, :KV_K].rearrange("(f t) -> f t", t=R)
            kvl_v = kv_local[0, KV_K:].rearrange("(t f) -> t f", f=C)
            kva_k = kv_all[:, :KV_K].rearrange("s (f t) -> s f t", t=R)
            kva_v = kv_all[:, KV_K:].rearrange("s (t f) -> s t f", f=C)

            # ---------------- Phase A: LN1 + QKV + AllGather ----------------
            with (
                tc.tile_pool(name="qkvp", bufs=1) as qkvp,
                tc.tile_pool(name="lnp", bufs=2) as lnp,
                tc.tile_pool(name="ps_ln", bufs=1, space="PSUM") as ps_ln,
                tc.tile_pool(name="ps_qkv", bufs=3, space="PSUM") as ps_qkv,
            ):
                wattn_sb = qkvp.tile([P, CT, 3 * C], bf16)
                nc.sync.dma_start(wattn_sb[:], wattn_d.rearrange("(o p) f -> p o f", p=P))
                xTs_sb = qkvp.tile([P, CT, R], f32)
                nc.sync.dma_start(xTs_sb[:], xTs_d.rearrange("(o p) t -> p o t", p=P))

                xln_s = qkvp.tile([P, CT, R], bf16)
                xln_q = qkvp.tile([P, CT, R], bf16)
                _ln_transposed(nc, tc, lnp, ps_ln, xTs_sb, xln_s, ones_sb,
                               ln1w_sb, ln1b_sb, apply_ln1)

                # K^T = W_k^T @ xln_s  -> [C, R], features 768:1536 of wattn
                kt_sb = qkvp.tile([P, CT, R], bf16)
                for f in range(CT):
                    ps = ps_qkv.tile([P, R], f32, tag="qk_ps")
                    for k in range(CT):
                        nc.tensor.matmul(
                            ps[:], wattn_sb[:, k, C + P * f : C + P * (f + 1)],
                            xln_s[:, k, :], start=(k == 0), stop=(k == CT - 1),
                        )
                    nc.vector.tensor_scalar(
                        kt_sb[:, f, :], ps[:], bqk_sb[:, CT + f : CT + f + 1], None,
                        mybir.AluOpType.add,
                    )
                    nc.sync.dma_start(kvl_k[P * f : P * (f + 1), :], kt_sb[:, f, :])

                # V = xln_s^T @ W_v -> [R, C] natural layout (features 1536:2304)
                v_sb = qkvp.tile([P, QT, C], bf16)
                for t in range(QT):
                    for hh in range(2):
                        ps = ps_qkv.tile([P, 384], f32, tag="v_ps")
                        for k in range(CT):
                            nc.tensor.matmul(
                                ps[:], xln_s[:, k, P * t : P * (t + 1)],
                                wattn_sb[:, k, 2 * C + 384 * hh : 2 * C + 384 * (hh + 1)],
                                start=(k == 0), stop=(k == CT - 1),
                            )
                        if apply_bv:
                            nc.vector.tensor_add(
                                v_sb[:, t, 384 * hh : 384 * (hh + 1)], ps[:],
                                bv_sb[:, 384 * hh : 384 * (hh + 1)],
                            )
                        else:
                            nc.vector.tensor_copy(
                                v_sb[:, t, 384 * hh : 384 * (hh + 1)], ps[:]
                            )
                    nc.sync.dma_start(kvl_v[P * t : P * (t + 1), :], v_sb[:, t, :])

                # Q^T for own (interleaved) query rows
                _ln_transposed(nc, tc, lnp, ps_ln, xTq_sb, xln_q, ones_sb,
                               ln1w_sb, ln1b_sb, apply_ln1)
                for f in range(CT):
                    ps = ps_qkv.tile([P, R], f32, tag="qk_ps")
                    for k in range(CT):
                        nc.tensor.matmul(
                            ps[:], wattn_sb[:, k, P * f : P * (f + 1)],
                            xln_q[:, k, :], start=(k == 0), stop=(k == CT - 1),
                        )
                    nc.vector.tensor_scalar(
                        q_sb[:, f, :], ps[:], bqk_sb[:, f : f + 1], None,
                        mybir.AluOpType.add,
                    )

                # prefetch W_fc during attention
                nc.sync.dma_start(wfc_sb[:], wfc_d.rearrange("(o p) f -> p o f", p=P))

                nc.gpsimd.collective_compute(
                    "AllGather",
                    mybir.AluOpType.bypass,
                    replica_groups=[list(range(NC))],
                    ins=[kv_local.opt()],
                    outs=[kv_all.opt()],
                )

            # ---------------- Phase B: attention ----------------
            with (
                tc.tile_pool(name="attnp", bufs=4) as attnp,
                tc.tile_pool(name="pp", bufs=3) as pp,
                tc.tile_pool(name="normp", bufs=2) as normp,
                tc.tile_pool(name="ps_s", bufs=2, space="PSUM") as ps_s,
                tc.tile_pool(name="ps_y", bufs=2, space="PSUM") as ps_y,
                tc.tile_pool(name="ps_rs", bufs=2, space="PSUM") as ps_rs,
            ):
                for pr in range(PAIRS):
                    y_ps = ps_y.tile([P, R], f32, tag="y")
                    rs_ps = ps_rs.tile([P, R], f32, tag="rs")
                    for k in range(KT):
                        m = k // 8
                        d = k % 8
                        N = P * (4 - m)
                        kt_t = attnp.tile([P, P], bf16, tag="kt")
                        nc.sync.dma_start(
                            kt_t[:], kva_k[k // 4, P * pr : P * (pr + 1), P * (k % 4) : P * (k % 4 + 1)]
                        )
                        vt_t = attnp.tile([P, P], bf16, tag="vt")
                        nc.sync.dma_start(
                            vt_t[:], kva_v[k // 4, P * (k % 4) : P * (k % 4 + 1), P * pr : P * (pr + 1)]
                        )
                        s_ps = ps_s.tile([P, 2, R], f32, tag="s")
                        nc.tensor.matmul(s_ps[:, 0, :N], kt_t[0:HD, :], q_sb[0:HD, pr, 0:N])
                        nc.tensor.matmul(s_ps[:, 1, :N], kt_t[HD:P, :], q_sb[HD:P, pr, 0:N])
                        p_t = pp.tile([P, 2, R], bf16, tag="p")
                        nc.scalar.activation(p_t[:, :, :N], s_ps[:, :, :N], AF.Exp, scale=0.125)
                        # causal mask on the diagonal 128-col group
                        nc.gpsimd.tensor_mul(
                            p_t[:, :, N - P : N],
                            p_t[:, :, N - P : N],
                            masks_sb[:, d : d + 1, :].to_broadcast((P, 2, P)),
                        )
                        nc.tensor.matmul(
                            y_ps[0:HD, 0:N], vt_t[:, 0:HD], p_t[:, 0, :N],
                            start=(k == 0), stop=(k == KT - 1),
                        )
                        nc.tensor.matmul(
                            y_ps[HD:P, 0:N], vt_t[:, HD:P], p_t[:, 1, :N],
                            start=(k == 0), stop=(k == KT - 1),
                        )
                        nc.tensor.matmul(
                            rs_ps[0:HD, 0:N], ones_sb[:, 0:HD], p_t[:, 0, :N],
                            start=(k == 0), stop=(k == KT - 1),
                        )
                        nc.tensor.matmul(
                            rs_ps[HD:P, 0:N], ones_sb[:, HD:P], p_t[:, 1, :N],
                            start=(k == 0), stop=(k == KT - 1),
                        )
                    recip = normp.tile([P, R], f32, tag="recip")
                    nc.vector.reciprocal(recip[:], rs_ps[:])
                    nc.vector.tensor_mul(ynorm_sb[:, pr, :], y_ps[:], recip[:])

            # ---------------- Phase C: proj + LN2 + MLP + out ----------------
            with (
                tc.tile_pool(name="mlpp", bufs=1) as mlpp,
                tc.tile_pool(name="lnp2", bufs=2) as lnp2,
            ):
                with (
                    tc.tile_pool(name="ps_proj", bufs=2, space="PSUM") as ps_proj,
                    tc.tile_pool(name="ps_ln2", bufs=1, space="PSUM") as ps_ln2,
                ):
                    for f in range(CT):
                        ps = ps_proj.tile([P, R], f32, tag="proj")
                        for k in range(CT):
                            nc.tensor.matmul(
                                ps[:], wproj_sb[:, k, P * f : P * (f + 1)],
                                ynorm_sb[:, k, :], start=(k == 0), stop=(k == CT - 1),
                            )
                        # z = (proj + b_proj) + x
                        nc.vector.scalar_tensor_tensor(
                            z_sb[:, f, :], ps[:], bproj_sb[:, f : f + 1], xTq_sb[:, f, :],
                            mybir.AluOpType.add, mybir.AluOpType.add,
                        )
                    _ln_transposed(nc, tc, lnp2, ps_ln2, z_sb, xln2_sb, ones_sb,
                                   ln2w_sb, ln2b_sb, apply_ln2)

                wfc2_sb = mlpp.tile([P, HT, C], bf16)
                nc.sync.dma_start(wfc2_sb[:], wfc2_d.rearrange("(o p) f -> p o f", p=P))
                h_sb = mlpp.tile([P, CT, R], bf16)
                with (
                    tc.tile_pool(name="ps_fc1", bufs=2, space="PSUM") as ps_fc1,
                    tc.tile_pool(name="ps_o", bufs=1, space="PSUM") as ps_o,
                ):
                    o_ps = [ps_o.tile([P, R], f32, tag=f"o{f}", name=f"o_ps{f}") for f in range(CT)]
                    for chunk in range(4):
                        for hf in range(CT):
                            hh = CT * chunk + hf
                            ps = ps_fc1.tile([P, R], f32, tag="fc1")
                            for k in range(CT):
                                nc.tensor.matmul(
                                    ps[:], wfc_sb[:, k, P * hh : P * (hh + 1)],
                                    xln2_sb[:, k, :], start=(k == 0), stop=(k == CT - 1),
                                )
                            nc.scalar.activation(
                                h_sb[:, hf, :], ps[:], AF.Gelu, bias=bfc_sb[:, hh : hh + 1]
                            )
                            for f in range(CT):
                                nc.tensor.matmul(
                                    o_ps[f][:], wfc2_sb[:, hh, P * f : P * (f + 1)],
                                    h_sb[:, hf, :], start=(hh == 0), stop=(hh == HT - 1),
                                )
                    outT_sb = mlpp.tile([P, CT, R], f32)
                    for f in range(CT):
                        nc.vector.scalar_tensor_tensor(
                            outT_sb[:, f, :], o_ps[f][:], bfc2_sb[:, f : f + 1], z_sb[:, f, :],
                            mybir.AluOpType.add, mybir.AluOpType.add,
                        )
                        nc.sync.dma_start(outT_d[P * f : P * (f + 1), :], outT_sb[:, f, :])

    nc.compile()
    _CACHE[key] = nc
    return nc


def _query_tokens(c):
    """Token ids owned by core c, in on-chip column order (j desc, i asc)."""
    return np.concatenate([1024 * j + 8 * np.arange(P) + c for j in (3, 2, 1, 0)])


def kernel(x, ln1_w, ln1_b, W_attn, b_attn, W_proj, b_proj,
           ln2_w, ln2_b, W_fc, b_fc, W_fc2, b_fc2):
    x = np.asarray(x, np.float32)
    ln1_w = np.asarray(ln1_w, np.float32)
    ln1_b = np.asarray(ln1_b, np.float32)
    W_attn = np.asarray(W_attn, np.float32)
    b_attn = np.asarray(b_attn, np.float32)
    W_proj = np.asarray(W_proj, np.float32)
    b_proj = np.asarray(b_proj, np.float32)
    ln2_w = np.asarray(ln2_w, np.float32)
    ln2_b = np.asarray(ln2_b, np.float32)
    W_fc = np.asarray(W_fc, np.float32)
    b_fc = np.asarray(b_fc, np.float32)
    W_fc2 = np.asarray(W_fc2, np.float32)
    b_fc2 = np.asarray(b_fc2, np.float32)

    apply_ln1 = not (np.all(ln1_w == 1.0) and np.all(ln1_b == 0.0))
    apply_ln2 = not (np.all(ln2_w == 1.0) and np.all(ln2_b == 0.0))
    apply_bv = bool(np.any(b_attn[2 * C :] != 0.0))

    nc = _build(apply_ln1, apply_ln2, apply_bv)

    xf = x[0]  # [T, C]
    wattn_b = W_attn.astype(BF16)
    wproj_b = W_proj.astype(BF16)
    wfc_b = W_fc.astype(BF16)
    wfc2_b = W_fc2.astype(BF16)
    bqk = np.ascontiguousarray(b_attn[: 2 * C].reshape(2 * CT, P).T)
    bproj = np.ascontiguousarray(b_proj.reshape(CT, P).T)
    bfc = np.ascontiguousarray(b_fc.reshape(HT, P).T)
    bfc2 = np.ascontiguousarray(b_fc2.reshape(CT, P).T)
    ones = np.ones((P, P), BF16)

    in_maps = []
    qtok = []
    for c in range(NC):
        xTs = np.ascontiguousarray(xf[R * c : R * (c + 1), :].T)
        qt = _query_tokens(c)
        qtok.append(qt)
        xTq = np.ascontiguousarray(xf[qt, :].T)
        kk = np.arange(P)[:, None, None]
        dd = np.arange(8)[None, :, None]
        ii = np.arange(P)[None, None, :]
        masks = ((8 * ii + c - 128 * dd - kk) >= 0).astype(BF16)
        m = {
            "xTs": xTs, "xTq": xTq, "masks": masks, "ones": ones,
            "wattn": wattn_b, "wproj": wproj_b, "wfc": wfc_b, "wfc2": wfc2_b,
            "bqk": bqk, "bproj": bproj, "bfc": bfc, "bfc2": bfc2,
        }
        if apply_bv:
            m["bv"] = np.ascontiguousarray(np.broadcast_to(b_attn[2 * C :], (P, C)))
        if apply_ln1:
            m["ln1w"] = np.ascontiguousarray(ln1_w.reshape(CT, P).T)
            m["ln1b"] = np.ascontiguousarray(ln1_b.reshape(CT, P).T)
        if apply_ln2:
            m["ln2w"] = np.ascontiguousarray(ln2_w.reshape(CT, P).T)
            m["ln2b"] = np.ascontiguousarray(ln2_b.reshape(CT, P).T)
        in_maps.append(m)

    res = run_bass_kernel_spmd(nc, in_maps, list(range(NC)))

    out = np.empty((T, C), np.float32)
    for c in range(NC):
        out[qtok[c], :] = res.results[c]["outT"].T
    return out[None, :, :]
